# revision 8
# baseline (speedup 1.0000x reference)
"""Trainium2 Bass kernel for submanifold sparse conv net (gnn_message_passing).

Network: mask = (x != 0); y = BN(x) masked; y1 = relu(subm_conv3x3(y, w1) + b1);
y2 = relu(subm_conv3x3(y1, w2) + b2); out = NCHW(y2).  B,H,W = 4,512,512, C: 1->32->64.

Sharding: H split into 8 slabs of 64 rows (one per NeuronCore), 2-row halo.

Per-core design (all per batch; pixel index p = row*516 + col, 2 zero-pad
cols each side; DRAM holds ONE padded x plane + ONE padded mask plane per
batch -- the 15 (dh,dw)-shifted rhs copies are materialized by a single DMA
with a 3-level partition access pattern):

- conv1: one matmul per y1 row: K=30 (2 planes x 3 dh x 5 dw shifted reads),
  M=97 = 3 "dw groups" x 32 channels + 1 mask-passthrough column: group g
  holds y1 evaluated at column+g-1, so conv2 needs only row shifts; column 96
  reproduces the center mask row so conv2's submanifold masking needs no
  separate DMA.
- conv2: TWO output rows per matmul (M=128 = 2 rows x 64 ch): 4 matmuls with
  K=97 (96 y1 partitions + mask row), one per y1 row-shift s in 0..3; lhsT_s
  carries w2[s] in cols 0:64 (row r) and w2[s-1] in cols 64:128 (row r+1).
  This is 2 matmuls/row instead of 3 and yields full-128-partition PSUM
  tiles, halving the relu/copy and output-DMA per-partition traffic.
- Masking (submanifold restrict) folded into matmuls via +LARGE*mask with
  bias -LARGE; BN folded into w1 host-side.
- bf16 matmul operands, fp32 PSUM; relu split across ACT/DVE/Pool engines.
"""

import sys

if "/opt/trn_rl_repo" not in sys.path:
    sys.path.insert(0, "/opt/trn_rl_repo")

import numpy as np
import ml_dtypes

BF16 = ml_dtypes.bfloat16

B, H, W = 4, 512, 512
NCORES = 8
ROWS = H // NCORES          # 64 output rows per core
CHUNK = 32                  # output rows per inner tile
NCHUNK = ROWS // CHUNK
SLAB = ROWS + 4             # 68 input rows incl 2-row halo each side
WP = W + 4                  # 516 padded cols
PLANE = SLAB * WP           # 35088
LROWS = CHUNK + 2           # y1 rows per chunk (1-row halo each side)
LFREE = LROWS * WP          # free elems per rhs1 chunk tile
YF = LROWS * W              # compact y1 free size
LARGE = 256.0
EPS = 1e-5

_cached = {}


def _build_nc():
    import concourse.bass as bass
    import concourse.mybir as mybir
    from concourse import bacc, tile

    f32 = mybir.dt.float32
    bf16 = mybir.dt.bfloat16
    AP = bass.AP
    Relu = mybir.ActivationFunctionType.Relu
    ADD = mybir.AluOpType.add
    MAX = mybir.AluOpType.max

    nc = bacc.Bacc("TRN2", target_bir_lowering=False, debug=False,
                   num_devices=NCORES)
    # +8 slack: the (plane,dh,dw)-shifted chunk reads run up to 4 elements
    # past the final mask plane (into never-used rhs columns >= 512)
    xm = nc.declare_dram_parameter("xm", [B * 2 * PLANE + 8], bf16, isOutput=False)
    wts = nc.declare_dram_parameter("wts", [97 * 640], bf16, isOutput=False)
    biasd = nc.declare_dram_parameter("biasd", [128 * 2], f32, isOutput=False)
    out = nc.declare_dram_parameter("out", [B * 64 * ROWS * W], f32, isOutput=True)

    with tile.TileContext(nc) as tc:
        with (
            tc.tile_pool(name="const", bufs=1) as cpool,
            tc.tile_pool(name="rhs1", bufs=2) as rpool,
            tc.tile_pool(name="y1", bufs=2) as ypool,
            tc.tile_pool(name="stage", bufs=3) as spool,
            tc.tile_pool(name="ps1", bufs=2, space="PSUM") as p1pool,
            tc.tile_pool(name="ps2", bufs=3, space="PSUM") as p2pool,
        ):
            wcat = cpool.tile([97, 640], bf16, tag="wcat")
            biasb = cpool.tile([128, 2], f32, tag="biasb")
            nc.sync.dma_start(out=wcat[:, :], in_=AP(wts, 0, [[640, 97], [1, 640]]))
            nc.sync.dma_start(out=biasb[:, :], in_=AP(biasd, 0, [[2, 128], [1, 2]]))
            w1t = wcat[0:30, 512:609]
            w2s = [wcat[0:97, 128 * s:128 * s + 128] for s in range(4)]
            bias1 = biasb[0:97, 0:1]
            bias2 = biasb[0:128, 1:2]
            # warm each compute engine against the const DMA lanes so the
            # first real ops don't exceed the per-instruction sync-wait limit
            scr = cpool.tile([128, 2], f32, tag="scratch")
            nc.scalar.activation(scr[0:97, 0:1], bias1, Relu, bias=bias1)
            nc.vector.tensor_scalar(scr[:, 1:2], biasb[:, 1:2], bias2, 0.0,
                                    op0=ADD, op1=MAX)

            for b in range(B):
                for k in range(NCHUNK):
                    # rhs1[(plane,dh,dw), rrl*516+c] = P[plane][32k+rrl+dh, c+dw]
                    # (one DMA, 3 partition levels over the two padded planes)
                    rhs1 = rpool.tile([30, LFREE], bf16, tag="rhs1")
                    for plane in range(2):
                        nc.sync.dma_start(
                            out=rhs1[15 * plane:15 * plane + 15, :],
                            in_=AP(xm,
                                   b * 2 * PLANE + plane * PLANE + CHUNK * k * WP,
                                   [[WP, 3], [1, 5], [1, LFREE]]),
                        )
                    # y1 compact: rows of 512, partitions 0:96 = 3 col-groups
                    # of 32 ch, partition 96 = center mask row
                    y1 = ypool.tile([97, YF], bf16, tag="y1")

                    def conv1_pair(j):
                        # y1 tile rows 2j, 2j+1
                        ps1 = p1pool.tile([97, 1024], f32, tag="ps1")
                        for half in range(2):
                            rrl = 2 * j + half
                            nc.tensor.matmul(
                                ps1[:, 512 * half:512 * half + 512], lhsT=w1t,
                                rhs=rhs1[:, rrl * WP:rrl * WP + 512],
                                start=True, stop=True,
                            )
                        dst = y1[0:97, 2 * j * W:2 * j * W + 1024]
                        if j % 8 == 7:  # GPSIMD can't read PSUM; split ACT/DVE
                            nc.vector.tensor_scalar(dst, ps1[:, :], bias1, 0.0,
                                                    op0=ADD, op1=MAX)
                        else:
                            nc.scalar.activation(dst, ps1[:, :], Relu, bias=bias1)

                    conv1_pair(0)
                    conv1_pair(1)
                    stage = None
                    for p in range(CHUNK // 2):
                        if p + 2 <= CHUNK // 2:
                            conv1_pair(p + 2)
                        ps2 = p2pool.tile([128, 512], f32, tag="ps2")
                        for s in range(4):
                            nc.tensor.matmul(
                                ps2[:, :], lhsT=w2s[s],
                                rhs=y1[0:97, (2 * p + s) * W:(2 * p + s) * W + 512],
                                start=(s == 0), stop=(s == 3),
                            )
                        q, t = divmod(p, 4)
                        if t == 0:
                            stage = spool.tile([128, 2048], f32, tag="stage")
                        dst = stage[:, 512 * t:512 * t + 512]
                        nc.vector.tensor_scalar(dst, ps2[:, :], bias2, 0.0,
                                                op0=ADD, op1=MAX)
                        if t == 3:
                            # 8 output rows (CHUNK*k + 8q .. +7), 64 channels;
                            # two DMAs (row parity) keep the APs at 3 dims
                            for rpar in range(2):
                                nc.sync.dma_start(
                                    out=AP(out,
                                           (b * 64 * ROWS + CHUNK * k + 8 * q
                                            + rpar) * W,
                                           [[ROWS * W, 64], [2 * W, 4], [1, W]]),
                                    in_=stage[64 * rpar:64 * rpar + 64, :],
                                )
    nc.finalize()
    return nc


def _prep_consts(bn_gamma, bn_beta, bn_mean, bn_var, w1, b1, w2, b2):
    s = float(bn_gamma[0] / np.sqrt(bn_var[0] + EPS))
    t = float(bn_beta[0] - bn_mean[0] * s)
    w1 = np.asarray(w1, np.float32)  # [3,3,1,32] (kh, kw, ci, co)
    w2 = np.asarray(w2, np.float32)  # [3,3,32,64]
    W1T = np.zeros((30, 97), np.float32)
    for plane in range(2):
        for dh in (-1, 0, 1):
            for dw in (-2, -1, 0, 1, 2):
                kp = plane * 15 + (dh + 1) * 5 + (dw + 2)
                for g in range(3):
                    dwp = dw - (g - 1)
                    col = slice(g * 32, g * 32 + 32)
                    if -1 <= dwp <= 1:
                        coef = s if plane == 0 else t
                        W1T[kp, col] += coef * w1[dh + 1, dwp + 1, 0, :]
                    if plane == 1 and dh == 0 and dw == (g - 1):
                        W1T[kp, col] += LARGE
    W1T[15 + 5 + 2, 96] = 1.0  # mask center passthrough
    W2g = np.zeros((3, 96, 64), np.float32)
    for dh in range(3):
        for g in range(3):
            W2g[dh, g * 32:g * 32 + 32] = w2[dh, g]
    wcat = np.zeros((97, 640), np.float32)
    for s4 in range(4):
        blk = np.zeros((97, 128), np.float32)
        if s4 <= 2:
            blk[0:96, 0:64] = W2g[s4]
        if s4 >= 1:
            blk[0:96, 64:128] = W2g[s4 - 1]
        if s4 == 1:
            blk[96, 0:64] = LARGE
        if s4 == 2:
            blk[96, 64:128] = LARGE
        wcat[:, 128 * s4:128 * s4 + 128] = blk
    wcat[0:30, 512:609] = W1T
    biasb = np.zeros((128, 2), np.float32)
    biasb[0:96, 0] = np.tile(np.asarray(b1, np.float32), 3) - LARGE
    biasb[96, 0] = 0.0
    biasb[:, 1] = np.tile(np.asarray(b2, np.float32), 2) - LARGE
    return wcat.ravel().astype(BF16), biasb.ravel().astype(np.float32)


def _prep_xm(x):
    """Per-core padded x+mask planes. x: [B,H,W,1] f32 -> 8 flat bf16 arrays."""
    x = np.asarray(x, np.float32)[..., 0]        # [B,H,W]
    mask = (x != 0.0).astype(np.float32)
    xg = np.zeros((B, H + 4, WP), np.float32)
    mg = np.zeros((B, H + 4, WP), np.float32)
    xg[:, 2:H + 2, 2:W + 2] = x
    mg[:, 2:H + 2, 2:W + 2] = mask
    maps = []
    for c in range(NCORES):
        r0 = c * ROWS
        xm = np.empty((B, 2, SLAB, WP), np.float32)
        xm[:, 0] = xg[:, r0:r0 + SLAB]
        xm[:, 1] = mg[:, r0:r0 + SLAB]
        maps.append(np.concatenate(
            [xm.ravel(), np.zeros(8, np.float32)]).astype(BF16))
    return maps


def kernel(x, bn_gamma, bn_beta, bn_mean, bn_var, w1, b1, w2, b2):
    from concourse.bass_utils import run_bass_kernel_spmd

    if "nc" not in _cached:
        _cached["nc"] = _build_nc()
    nc = _cached["nc"]
    wts, biasb = _prep_consts(bn_gamma, bn_beta, bn_mean, bn_var, w1, b1, w2, b2)
    xms = _prep_xm(x)
    in_maps = [{"xm": xms[c], "wts": wts, "biasd": biasb} for c in range(NCORES)]
    res = run_bass_kernel_spmd(nc, in_maps, list(range(NCORES)))
    full = np.empty((B, 64, H, W), np.float32)
    for c in range(NCORES):
        full[:, :, c * ROWS:(c + 1) * ROWS, :] = (
            np.asarray(res.results[c]["out"], np.float32).reshape(B, 64, ROWS, W))
    return full


# revision 65
# speedup vs baseline: 1.4709x; 1.4709x over previous
"""Trainium2 Bass kernel for submanifold sparse conv net (gnn_message_passing).

Network: mask = (x != 0); y = BN(x) masked; y1 = relu(subm_conv3x3(y, w1) + b1);
y2 = relu(subm_conv3x3(y1, w2) + b2); out = NCHW(y2).  B,H,W = 4,512,512, C: 1->32->64.

Sharding: H split into 8 slabs of 64 rows (one per NeuronCore), 2-row halo.

Per-core design (all per batch; pixel index p = row*516 + col, 2 zero-pad
cols each side; DRAM holds ONE padded x plane + ONE padded mask plane per
batch -- the 15 (dh,dw)-shifted rhs copies are materialized by a single DMA
with a 3-level partition access pattern):

- conv1: one matmul per y1 row: K=30 (2 planes x 3 dh x 5 dw shifted reads),
  M=97 = 3 "dw groups" x 32 channels + 1 mask-passthrough column: group g
  holds y1 evaluated at column+g-1, so conv2 needs only row shifts; column 96
  reproduces the center mask row so conv2's submanifold masking needs no
  separate DMA.
- conv2: TWO output rows per matmul (M=128 = 2 rows x 64 ch): 4 matmuls with
  K=97 (96 y1 partitions + mask row), one per y1 row-shift s in 0..3; lhsT_s
  carries w2[s] in cols 0:64 (row r) and w2[s-1] in cols 64:128 (row r+1).
  This is 2 matmuls/row instead of 3 and yields full-128-partition PSUM
  tiles, halving the relu/copy and output-DMA per-partition traffic.
- Masking (submanifold restrict) folded into matmuls via +LARGE*mask with
  bias -LARGE; BN folded into w1 host-side.
- bf16 matmul operands, fp32 PSUM; relus alternate ACT/DVE (GPSIMD cannot
  read PSUM). Conv1 pairs stream a few rows ahead of conv2 groups across
  chunk boundaries; rhs loads ride the idle Pool/SWDGE queue in 3 pieces
  per plane, out-DMAs are single-row [64,512] on SP/HWDGE (finer grain
  lets input loads slot between them), the final chunk drains odd rows
  via Pool. All knob values (pool bufs, LEAD, split points) were tuned
  against TimelineSim; PE ends ~92% busy at ~183 us vs the ~167 us
  784-matmul floor.
"""

import sys

if "/opt/trn_rl_repo" not in sys.path:
    sys.path.insert(0, "/opt/trn_rl_repo")

import numpy as np
import ml_dtypes

BF16 = ml_dtypes.bfloat16

B, H, W = 4, 512, 512
NCORES = 8
ROWS = H // NCORES          # 64 output rows per core
CHUNK = 32                  # output rows per inner tile
NCHUNK = ROWS // CHUNK
SLAB = ROWS + 4             # 68 input rows incl 2-row halo each side
WP = W + 4                  # 516 padded cols
PLANE = SLAB * WP           # 35088
LROWS = CHUNK + 2           # y1 rows per chunk (1-row halo each side)
LFREE = LROWS * WP          # free elems per rhs1 chunk tile
YF = LROWS * W              # compact y1 free size
LARGE = 256.0
EPS = 1e-5

_cached = {}


def _build_nc(nkm=5):
    # nkm: mask k-rows in conv1's contraction. 5 when the folded BN shift
    # t == 0 (mask taps only carry the LARGE center terms at dh=0), 15 for
    # the general case.
    import concourse.bass as bass
    import concourse.mybir as mybir
    from concourse import bacc, tile

    KC1 = 15 + nkm

    f32 = mybir.dt.float32
    bf16 = mybir.dt.bfloat16
    AP = bass.AP
    Relu = mybir.ActivationFunctionType.Relu
    ADD = mybir.AluOpType.add
    MAX = mybir.AluOpType.max

    nc = bacc.Bacc("TRN2", target_bir_lowering=False, debug=False,
                   num_devices=NCORES)
    # +8 slack: the (plane,dh,dw)-shifted chunk reads run up to 4 elements
    # past the final mask plane (into never-used rhs columns >= 512)
    xm = nc.declare_dram_parameter("xm", [B * 2 * PLANE + 8], bf16, isOutput=False)
    wts = nc.declare_dram_parameter("wts", [97 * 640], bf16, isOutput=False)
    biasd = nc.declare_dram_parameter("biasd", [128 * 2], f32, isOutput=False)
    out = nc.declare_dram_parameter("out", [B * 64 * ROWS * W], f32, isOutput=True)

    with tile.TileContext(nc) as tc:
        with (
            tc.tile_pool(name="const", bufs=1) as cpool,
            tc.tile_pool(name="rhs1", bufs=3) as rpool,
            tc.tile_pool(name="y1", bufs=2) as ypool,
            tc.tile_pool(name="stage", bufs=17) as spool,
            tc.tile_pool(name="ps1", bufs=3, space="PSUM") as p1pool,
            tc.tile_pool(name="ps2", bufs=2, space="PSUM") as p2pool,
        ):
            wcat = cpool.tile([97, 640], bf16, tag="wcat")
            biasb = cpool.tile([128, 2], f32, tag="biasb")
            w1t = wcat[0:KC1, 512:609]
            w2s = [wcat[0:97, 128 * s:128 * s + 128] for s in range(4)]
            bias1 = biasb[0:97, 0:1]
            bias2 = biasb[0:128, 1:2]
            # warm each compute engine against the const DMA lanes so the
            # first real ops don't exceed the per-instruction sync-wait limit
            scr = cpool.tile([128, 2], f32, tag="scratch")
            nc.scalar.activation(scr[0:97, 0:1], bias1, Relu, bias=bias1)
            nc.vector.tensor_scalar(scr[:, 1:2], biasb[:, 1:2], bias2, 0.0,
                                    op0=ADD, op1=MAX)

            chunks = [(b, k) for b in range(B) for k in range(NCHUNK)]
            NC = len(chunks)
            rtiles = {}
            ytiles = {}
            FA = 18 * WP  # row split so early conv1 pairs start after half a DMA

            def load_rhs(ci):
                # rhs1[(plane,dh,dw), rrl*516+c] = P[plane][32k+rrl+dh, c+dw]
                # (3 partition levels over each padded plane; 2 halves per
                # plane, both planes' first halves first, so the first rows
                # only wait on the leading two transfers)
                bb, kk = chunks[ci]
                rhs1 = rpool.tile([KC1, LFREE], bf16, name=f"rhs_{ci}",
                                  tag="rhs1")
                # both planes' first halves first, so the first pairs only
                # wait on the leading two transfers; issued from the idle
                # Pool engine so they never sit behind out-DMAs on SP
                for f0, f1 in ((0, 10 * WP), (10 * WP, FA), (FA, LFREE)):
                    xbase = bb * 2 * PLANE + CHUNK * kk * WP
                    nc.gpsimd.dma_start(
                        out=rhs1[0:15, f0:f1],
                        in_=AP(xm, xbase + f0, [[WP, 3], [1, 5], [1, f1 - f0]]),
                    )
                    if nkm == 15:
                        nc.gpsimd.dma_start(
                            out=rhs1[15:30, f0:f1],
                            in_=AP(xm, xbase + PLANE + f0,
                                   [[WP, 3], [1, 5], [1, f1 - f0]]),
                        )
                    else:
                        nc.gpsimd.dma_start(
                            out=rhs1[15:20, f0:f1],
                            in_=AP(xm, xbase + PLANE + WP + f0,
                                   [[1, 5], [1, f1 - f0]]),
                        )
                rtiles[ci] = rhs1

            def conv1_pair(ci, j):
                # y1 tile rows 2j, 2j+1 of chunk ci
                if j == 0:
                    # y1 compact: rows of 512, partitions 0:96 = 3 col-groups
                    # of 32 ch, partition 96 = center mask row
                    ytiles[ci] = ypool.tile([97, YF], bf16, name=f"y1_{ci}",
                                            tag="y1")
                rhs1 = rtiles[ci]
                y1 = ytiles[ci]
                ps1 = p1pool.tile([97, 1024], f32, tag="ps1")
                for half in range(2):
                    rrl = 2 * j + half
                    nc.tensor.matmul(
                        ps1[:, 512 * half:512 * half + 512], lhsT=w1t,
                        rhs=rhs1[:, rrl * WP:rrl * WP + 512],
                        start=True, stop=True,
                    )
                dst = y1[0:97, 2 * j * W:2 * j * W + 1024]
                if j % 2 == 1:  # GPSIMD can't read PSUM; alternate ACT/DVE
                    nc.scalar.activation(dst, ps1[:, :], Relu, bias=bias1)
                else:
                    nc.vector.tensor_scalar(dst, ps1[:, :], bias1, 0.0,
                                            op0=ADD, op1=MAX)

            # global software pipeline: the conv1 pair stream leads the conv2
            # group stream, flowing across chunk boundaries
            pair_list = [(ci, j) for ci in range(NC) for j in range(LROWS // 2)]
            emitted = [0]
            LEAD = 4  # in rows

            def emit_rows_until(target):
                # target is in row units; pairs cover 2 rows each
                while 2 * emitted[0] < min(target, 2 * len(pair_list)):
                    ci, j = pair_list[emitted[0]]
                    conv1_pair(ci, j)
                    emitted[0] += 1

            # chunk 0's leading x-half rides SP so its descriptor generation
            # runs in parallel with Pool's; consts follow on SP
            rhs0 = rpool.tile([KC1, LFREE], bf16, name="rhs_0", tag="rhs1")
            nc.sync.dma_start(
                out=rhs0[0:15, 0:FA],
                in_=AP(xm, 0, [[WP, 3], [1, 5], [1, FA]]),
            )
            if nkm == 15:
                nc.gpsimd.dma_start(
                    out=rhs0[15:30, 0:FA],
                    in_=AP(xm, PLANE, [[WP, 3], [1, 5], [1, FA]]),
                )
            else:
                nc.gpsimd.dma_start(
                    out=rhs0[15:20, 0:FA],
                    in_=AP(xm, PLANE + WP, [[1, 5], [1, FA]]),
                )
            nc.sync.dma_start(out=wcat[:, :], in_=AP(wts, 0, [[640, 97], [1, 640]]))
            nc.sync.dma_start(out=biasb[:, :], in_=AP(biasd, 0, [[2, 128], [1, 2]]))
            for f0, f1 in ((FA, LFREE),):
                nc.gpsimd.dma_start(
                    out=rhs0[0:15, f0:f1],
                    in_=AP(xm, f0, [[WP, 3], [1, 5], [1, f1 - f0]]),
                )
                if nkm == 15:
                    nc.gpsimd.dma_start(
                        out=rhs0[15:30, f0:f1],
                        in_=AP(xm, PLANE + f0,
                               [[WP, 3], [1, 5], [1, f1 - f0]]),
                    )
                else:
                    nc.gpsimd.dma_start(
                        out=rhs0[15:20, f0:f1],
                        in_=AP(xm, PLANE + WP + f0,
                               [[1, 5], [1, f1 - f0]]),
                    )
            rtiles[0] = rhs0
            load_rhs(1)
            for ci, (b, k) in enumerate(chunks):
                if ci + 2 < NC:
                    # prefetch two chunks ahead at chunk top (the row stream
                    # for chunk ci+1 starts now), ahead of this chunk's
                    # out-DMAs in the in-order SP queue
                    load_rhs(ci + 2)
                rtiles.pop(ci - 2, None)
                ytiles.pop(ci - 2, None)
                emit_rows_until(ci * LROWS + 4 + LEAD)
                y1 = ytiles[ci]
                for p in range(CHUNK // 2):
                    ps2 = p2pool.tile([128, 512], f32, tag="ps2")
                    for s in range(4):
                        nc.tensor.matmul(
                            ps2[:, :], lhsT=w2s[s],
                            rhs=y1[0:97, (2 * p + s) * W:(2 * p + s) * W + 512],
                            start=(s == 0), stop=(s == 3),
                        )
                    stage = spool.tile([128, 512], f32, tag="stage")
                    if p % 2 == 0:
                        nc.vector.tensor_scalar(stage[:, :], ps2[:, :], bias2,
                                                0.0, op0=ADD, op1=MAX)
                    else:
                        nc.scalar.activation(stage[:, :], ps2[:, :], Relu,
                                             bias=bias2)
                    # 2 output rows (CHUNK*k + 2p, +1), 64 channels; two
                    # single-row DMAs so input loads can slot in between.
                    # In the final chunk odd rows drain via the idle Pool
                    # SWDGE path to halve the tail flush.
                    for rpar in range(2):
                        eng = (nc.gpsimd if rpar == 1 and ci == NC - 1
                               else nc.sync)
                        eng.dma_start(
                            out=AP(out,
                                   (b * 64 * ROWS + CHUNK * k + 2 * p
                                    + rpar) * W,
                                   [[ROWS * W, 64], [1, W]]),
                            in_=stage[64 * rpar:64 * rpar + 64, :],
                        )
                    emit_rows_until(ci * LROWS + 2 * p + 6 + LEAD)
    nc.finalize()
    return nc


def _prep_consts(bn_gamma, bn_beta, bn_mean, bn_var, w1, b1, w2, b2):
    s = float(bn_gamma[0] / np.sqrt(bn_var[0] + EPS))
    t = float(bn_beta[0] - bn_mean[0] * s)
    w1 = np.asarray(w1, np.float32)  # [3,3,1,32] (kh, kw, ci, co)
    w2 = np.asarray(w2, np.float32)  # [3,3,32,64]
    nkm = 5 if t == 0.0 else 15
    W1T = np.zeros((15 + nkm, 97), np.float32)
    for plane in range(2):
        for dh in (-1, 0, 1):
            if plane == 1 and nkm == 5 and dh != 0:
                continue
            for dw in (-2, -1, 0, 1, 2):
                if plane == 0:
                    kp = (dh + 1) * 5 + (dw + 2)
                else:
                    kp = 15 + ((dh + 1) * 5 if nkm == 15 else 0) + (dw + 2)
                for g in range(3):
                    dwp = dw - (g - 1)
                    col = slice(g * 32, g * 32 + 32)
                    if -1 <= dwp <= 1:
                        coef = s if plane == 0 else t
                        if plane == 0 or nkm == 15:
                            W1T[kp, col] += coef * w1[dh + 1, dwp + 1, 0, :]
                    if plane == 1 and dh == 0 and dw == (g - 1):
                        W1T[kp, col] += LARGE
    # mask center passthrough -> y1 partition 96
    W1T[15 + (5 if nkm == 15 else 0) + 2, 96] = 1.0
    W2g = np.zeros((3, 96, 64), np.float32)
    for dh in range(3):
        for g in range(3):
            W2g[dh, g * 32:g * 32 + 32] = w2[dh, g]
    wcat = np.zeros((97, 640), np.float32)
    for s4 in range(4):
        blk = np.zeros((97, 128), np.float32)
        if s4 <= 2:
            blk[0:96, 0:64] = W2g[s4]
        if s4 >= 1:
            blk[0:96, 64:128] = W2g[s4 - 1]
        if s4 == 1:
            blk[96, 0:64] = LARGE
        if s4 == 2:
            blk[96, 64:128] = LARGE
        wcat[:, 128 * s4:128 * s4 + 128] = blk
    wcat[0:15 + nkm, 512:609] = W1T
    biasb = np.zeros((128, 2), np.float32)
    biasb[0:96, 0] = np.tile(np.asarray(b1, np.float32), 3) - LARGE
    biasb[96, 0] = 0.0
    biasb[:, 1] = np.tile(np.asarray(b2, np.float32), 2) - LARGE
    return wcat.ravel().astype(BF16), biasb.ravel().astype(np.float32), nkm


def _prep_xm(x):
    """Per-core padded x+mask planes. x: [B,H,W,1] f32 -> 8 flat bf16 arrays."""
    x = np.asarray(x, np.float32)[..., 0]        # [B,H,W]
    mask = (x != 0.0).astype(np.float32)
    xg = np.zeros((B, H + 4, WP), np.float32)
    mg = np.zeros((B, H + 4, WP), np.float32)
    xg[:, 2:H + 2, 2:W + 2] = x
    mg[:, 2:H + 2, 2:W + 2] = mask
    maps = []
    for c in range(NCORES):
        r0 = c * ROWS
        xm = np.empty((B, 2, SLAB, WP), np.float32)
        xm[:, 0] = xg[:, r0:r0 + SLAB]
        xm[:, 1] = mg[:, r0:r0 + SLAB]
        maps.append(np.concatenate(
            [xm.ravel(), np.zeros(8, np.float32)]).astype(BF16))
    return maps


def kernel(x, bn_gamma, bn_beta, bn_mean, bn_var, w1, b1, w2, b2):
    from concourse.bass_utils import run_bass_kernel_spmd

    wts, biasb, nkm = _prep_consts(bn_gamma, bn_beta, bn_mean, bn_var,
                                   w1, b1, w2, b2)
    if ("nc", nkm) not in _cached:
        _cached[("nc", nkm)] = _build_nc(nkm)
    _cached["nc"] = nc = _cached[("nc", nkm)]
    xms = _prep_xm(x)
    in_maps = [{"xm": xms[c], "wts": wts, "biasd": biasb} for c in range(NCORES)]
    res = run_bass_kernel_spmd(nc, in_maps, list(range(NCORES)))
    full = np.empty((B, 64, H, W), np.float32)
    for c in range(NCORES):
        full[:, :, c * ROWS:(c + 1) * ROWS, :] = (
            np.asarray(res.results[c]["out"], np.float32).reshape(B, 64, ROWS, W))
    return full



# revision 74
# speedup vs baseline: 1.4722x; 1.0009x over previous
"""Trainium2 Bass kernel for submanifold sparse conv net (gnn_message_passing).

Network: mask = (x != 0); y = BN(x) masked; y1 = relu(subm_conv3x3(y, w1) + b1);
y2 = relu(subm_conv3x3(y1, w2) + b2); out = NCHW(y2).  B,H,W = 4,512,512, C: 1->32->64.

Sharding: H split into 8 slabs of 64 rows (one per NeuronCore), 2-row halo.

Per-core design (all per batch; pixel index p = row*516 + col, 2 zero-pad
cols each side; DRAM holds ONE padded x plane + ONE padded mask plane per
batch -- the 15 (dh,dw)-shifted rhs copies are materialized by a single DMA
with a 3-level partition access pattern):

- conv1: one matmul per y1 row: K=30 (2 planes x 3 dh x 5 dw shifted reads),
  M=97 = 3 "dw groups" x 32 channels + 1 mask-passthrough column: group g
  holds y1 evaluated at column+g-1, so conv2 needs only row shifts; column 96
  reproduces the center mask row so conv2's submanifold masking needs no
  separate DMA.
- conv2: TWO output rows per matmul (M=128 = 2 rows x 64 ch): 4 matmuls with
  K=97 (96 y1 partitions + mask row), one per y1 row-shift s in 0..3; lhsT_s
  carries w2[s] in cols 0:64 (row r) and w2[s-1] in cols 64:128 (row r+1).
  This is 2 matmuls/row instead of 3 and yields full-128-partition PSUM
  tiles, halving the relu/copy and output-DMA per-partition traffic.
- Masking (submanifold restrict) folded into matmuls via +LARGE*mask with
  bias -LARGE; BN folded into w1 host-side.
- bf16 matmul operands, fp32 PSUM; relus alternate ACT/DVE (GPSIMD cannot
  read PSUM). Conv1 pairs stream a few rows ahead of conv2 groups across
  chunk boundaries; rhs loads ride the idle Pool/SWDGE queue in 3 pieces
  per plane, out-DMAs are single-row [64,512] on SP/HWDGE (finer grain
  lets input loads slot between them), the final chunk drains odd rows
  via Pool. All knob values (pool bufs, LEAD, split points) were tuned
  against TimelineSim; PE ends ~92% busy at ~183 us vs the ~167 us
  784-matmul floor.
"""

import sys

if "/opt/trn_rl_repo" not in sys.path:
    sys.path.insert(0, "/opt/trn_rl_repo")

import numpy as np
import ml_dtypes

BF16 = ml_dtypes.bfloat16

B, H, W = 4, 512, 512
NCORES = 8
ROWS = H // NCORES          # 64 output rows per core
CHUNK = 32                  # output rows per inner tile
NCHUNK = ROWS // CHUNK
SLAB = ROWS + 4             # 68 input rows incl 2-row halo each side
WP = W + 4                  # 516 padded cols
PLANE = SLAB * WP           # 35088
LROWS = CHUNK + 2           # y1 rows per chunk (1-row halo each side)
LFREE = LROWS * WP          # free elems per rhs1 chunk tile
YF = LROWS * W              # compact y1 free size
LARGE = 256.0
EPS = 1e-5

_cached = {}


def _build_nc(nkm=5):
    # nkm: mask k-rows in conv1's contraction. 5 when the folded BN shift
    # t == 0 (mask taps only carry the LARGE center terms at dh=0), 15 for
    # the general case.
    import concourse.bass as bass
    import concourse.mybir as mybir
    from concourse import bacc, tile

    KC1 = 15 + nkm

    f32 = mybir.dt.float32
    bf16 = mybir.dt.bfloat16
    AP = bass.AP
    Relu = mybir.ActivationFunctionType.Relu
    ADD = mybir.AluOpType.add
    MAX = mybir.AluOpType.max

    nc = bacc.Bacc("TRN2", target_bir_lowering=False, debug=False,
                   num_devices=NCORES)
    # +8 slack: the (plane,dh,dw)-shifted chunk reads run up to 4 elements
    # past the final mask plane (into never-used rhs columns >= 512)
    xm = nc.declare_dram_parameter("xm", [B * 2 * PLANE + 8], bf16, isOutput=False)
    wts = nc.declare_dram_parameter("wts", [97 * 640], bf16, isOutput=False)
    biasd = nc.declare_dram_parameter("biasd", [128 * 2], f32, isOutput=False)
    out = nc.declare_dram_parameter("out", [B * 64 * ROWS * W], f32, isOutput=True)

    with tile.TileContext(nc) as tc:
        with (
            tc.tile_pool(name="const", bufs=1) as cpool,
            tc.tile_pool(name="rhs1", bufs=3) as rpool,
            tc.tile_pool(name="y1", bufs=2) as ypool,
            tc.tile_pool(name="stage", bufs=17) as spool,
            tc.tile_pool(name="ps1", bufs=3, space="PSUM") as p1pool,
            tc.tile_pool(name="ps2", bufs=2, space="PSUM") as p2pool,
        ):
            wcat = cpool.tile([97, 640], bf16, tag="wcat")
            biasb = cpool.tile([128, 2], f32, tag="biasb")
            w1t = wcat[0:KC1, 512:609]
            w2s = [wcat[0:97, 128 * s:128 * s + 128] for s in range(4)]
            bias1 = biasb[0:97, 0:1]
            bias2 = biasb[0:128, 1:2]
            # warm each compute engine against the const DMA lanes so the
            # first real ops don't exceed the per-instruction sync-wait limit
            scr = cpool.tile([128, 2], f32, tag="scratch")
            nc.scalar.activation(scr[0:97, 0:1], bias1, Relu, bias=bias1)
            nc.vector.tensor_scalar(scr[:, 1:2], biasb[:, 1:2], bias2, 0.0,
                                    op0=ADD, op1=MAX)

            chunks = [(b, k) for b in range(B) for k in range(NCHUNK)]
            NC = len(chunks)
            rtiles = {}
            ytiles = {}
            FA = 18 * WP  # row split so early conv1 pairs start after half a DMA

            def load_rhs(ci):
                # rhs1[(plane,dh,dw), rrl*516+c] = P[plane][32k+rrl+dh, c+dw]
                # (3 partition levels over each padded plane; 2 halves per
                # plane, both planes' first halves first, so the first rows
                # only wait on the leading two transfers)
                bb, kk = chunks[ci]
                rhs1 = rpool.tile([KC1, LFREE], bf16, name=f"rhs_{ci}",
                                  tag="rhs1")
                # both planes' first halves first, so the first pairs only
                # wait on the leading two transfers; issued from the idle
                # Pool engine so they never sit behind out-DMAs on SP
                fend = LFREE if kk == 0 else 32 * WP
                for f0, f1 in ((0, 10 * WP), (10 * WP, FA), (FA, fend)):
                    # k=1 windows shift down 2 rows (tile row 0 = y1 row 33)
                    xbase = bb * 2 * PLANE + (CHUNK + 2) * kk * WP
                    nc.gpsimd.dma_start(
                        out=rhs1[0:15, f0:f1],
                        in_=AP(xm, xbase + f0, [[WP, 3], [1, 5], [1, f1 - f0]]),
                    )
                    if nkm == 15:
                        nc.gpsimd.dma_start(
                            out=rhs1[15:30, f0:f1],
                            in_=AP(xm, xbase + PLANE + f0,
                                   [[WP, 3], [1, 5], [1, f1 - f0]]),
                        )
                    else:
                        nc.gpsimd.dma_start(
                            out=rhs1[15:20, f0:f1],
                            in_=AP(xm, xbase + PLANE + WP + f0,
                                   [[1, 5], [1, f1 - f0]]),
                        )
                rtiles[ci] = rhs1

            def conv1_pair(ci, j):
                # y1 tile rows 2j, 2j+1 of chunk ci
                if j == 0:
                    # y1 compact: rows of 512, partitions 0:96 = 3 col-groups
                    # of 32 ch, partition 96 = center mask row
                    ytiles[ci] = ypool.tile([97, YF], bf16, name=f"y1_{ci}",
                                            tag="y1")
                rhs1 = rtiles[ci]
                y1 = ytiles[ci]
                ps1 = p1pool.tile([97, 1024], f32, tag="ps1")
                for half in range(2):
                    rrl = 2 * j + half
                    nc.tensor.matmul(
                        ps1[:, 512 * half:512 * half + 512], lhsT=w1t,
                        rhs=rhs1[:, rrl * WP:rrl * WP + 512],
                        start=True, stop=True,
                    )
                dst = y1[0:97, 2 * j * W:2 * j * W + 1024]
                if j % 2 == 1:  # GPSIMD can't read PSUM; alternate ACT/DVE
                    nc.scalar.activation(dst, ps1[:, :], Relu, bias=bias1)
                else:
                    nc.vector.tensor_scalar(dst, ps1[:, :], bias1, 0.0,
                                            op0=ADD, op1=MAX)

            # global software pipeline: the conv1 pair stream leads the conv2
            # group stream, flowing across chunk boundaries. k=1 chunks have
            # one pair fewer: their first group reads its two upper y1 rows
            # from the previous chunk's tile (no halo recompute).
            NP = [17 if kk == 0 else 16 for (_, kk) in chunks]
            row_start = [0]
            for n in NP:
                row_start.append(row_start[-1] + 2 * n)
            pair_list = [(ci, j) for ci in range(NC) for j in range(NP[ci])]
            emitted = [0]
            LEAD = 4  # in rows

            def emit_rows_until(target):
                # target is in row units; pairs cover 2 rows each
                while 2 * emitted[0] < min(target, 2 * len(pair_list)):
                    ci, j = pair_list[emitted[0]]
                    conv1_pair(ci, j)
                    emitted[0] += 1

            # chunk 0's leading x-half rides SP so its descriptor generation
            # runs in parallel with Pool's; consts follow on SP
            rhs0 = rpool.tile([KC1, LFREE], bf16, name="rhs_0", tag="rhs1")
            nc.sync.dma_start(
                out=rhs0[0:15, 0:FA],
                in_=AP(xm, 0, [[WP, 3], [1, 5], [1, FA]]),
            )
            if nkm == 15:
                nc.gpsimd.dma_start(
                    out=rhs0[15:30, 0:FA],
                    in_=AP(xm, PLANE, [[WP, 3], [1, 5], [1, FA]]),
                )
            else:
                nc.gpsimd.dma_start(
                    out=rhs0[15:20, 0:FA],
                    in_=AP(xm, PLANE + WP, [[1, 5], [1, FA]]),
                )
            nc.sync.dma_start(out=wcat[:, :], in_=AP(wts, 0, [[640, 97], [1, 640]]))
            nc.sync.dma_start(out=biasb[:, :], in_=AP(biasd, 0, [[2, 128], [1, 2]]))
            for f0, f1 in ((FA, LFREE),):
                nc.gpsimd.dma_start(
                    out=rhs0[0:15, f0:f1],
                    in_=AP(xm, f0, [[WP, 3], [1, 5], [1, f1 - f0]]),
                )
                if nkm == 15:
                    nc.gpsimd.dma_start(
                        out=rhs0[15:30, f0:f1],
                        in_=AP(xm, PLANE + f0,
                               [[WP, 3], [1, 5], [1, f1 - f0]]),
                    )
                else:
                    nc.gpsimd.dma_start(
                        out=rhs0[15:20, f0:f1],
                        in_=AP(xm, PLANE + WP + f0,
                               [[1, 5], [1, f1 - f0]]),
                    )
            rtiles[0] = rhs0
            load_rhs(1)
            for ci, (b, k) in enumerate(chunks):
                if ci + 2 < NC:
                    # prefetch two chunks ahead at chunk top (the row stream
                    # for chunk ci+1 starts now), ahead of this chunk's
                    # out-DMAs in the in-order SP queue
                    load_rhs(ci + 2)
                rtiles.pop(ci - 2, None)
                ytiles.pop(ci - 2, None)
                emit_rows_until(row_start[ci] + 4 + LEAD)
                y1 = ytiles[ci]
                for p in range(CHUNK // 2):
                    ps2 = p2pool.tile([128, 512], f32, tag="ps2")
                    for s in range(4):
                        if k == 0:
                            ysrc, t = y1, 2 * p + s
                        elif p == 0 and s < 2:
                            # top rows live in the previous chunk's tile
                            ysrc, t = ytiles[ci - 1], 32 + s
                        else:
                            ysrc, t = y1, 2 * p + s - 2
                        nc.tensor.matmul(
                            ps2[:, :], lhsT=w2s[s],
                            rhs=ysrc[0:97, t * W:t * W + 512],
                            start=(s == 0), stop=(s == 3),
                        )
                    stage = spool.tile([128, 512], f32, tag="stage")
                    if p % 2 == 0:
                        nc.vector.tensor_scalar(stage[:, :], ps2[:, :], bias2,
                                                0.0, op0=ADD, op1=MAX)
                    else:
                        nc.scalar.activation(stage[:, :], ps2[:, :], Relu,
                                             bias=bias2)
                    # 2 output rows (CHUNK*k + 2p, +1), 64 channels; two
                    # single-row DMAs so input loads can slot in between.
                    # In the final chunk odd rows drain via the idle Pool
                    # SWDGE path to halve the tail flush.
                    for rpar in range(2):
                        eng = (nc.gpsimd if rpar == 1 and ci == NC - 1
                               else nc.sync)
                        eng.dma_start(
                            out=AP(out,
                                   (b * 64 * ROWS + CHUNK * k + 2 * p
                                    + rpar) * W,
                                   [[ROWS * W, 64], [1, W]]),
                            in_=stage[64 * rpar:64 * rpar + 64, :],
                        )
                    emit_rows_until(row_start[ci] + 2 * p + 6 + LEAD)
    nc.finalize()
    return nc


def _prep_consts(bn_gamma, bn_beta, bn_mean, bn_var, w1, b1, w2, b2):
    s = float(bn_gamma[0] / np.sqrt(bn_var[0] + EPS))
    t = float(bn_beta[0] - bn_mean[0] * s)
    w1 = np.asarray(w1, np.float32)  # [3,3,1,32] (kh, kw, ci, co)
    w2 = np.asarray(w2, np.float32)  # [3,3,32,64]
    nkm = 5 if t == 0.0 else 15
    W1T = np.zeros((15 + nkm, 97), np.float32)
    for plane in range(2):
        for dh in (-1, 0, 1):
            if plane == 1 and nkm == 5 and dh != 0:
                continue
            for dw in (-2, -1, 0, 1, 2):
                if plane == 0:
                    kp = (dh + 1) * 5 + (dw + 2)
                else:
                    kp = 15 + ((dh + 1) * 5 if nkm == 15 else 0) + (dw + 2)
                for g in range(3):
                    dwp = dw - (g - 1)
                    col = slice(g * 32, g * 32 + 32)
                    if -1 <= dwp <= 1:
                        coef = s if plane == 0 else t
                        if plane == 0 or nkm == 15:
                            W1T[kp, col] += coef * w1[dh + 1, dwp + 1, 0, :]
                    if plane == 1 and dh == 0 and dw == (g - 1):
                        W1T[kp, col] += LARGE
    # mask center passthrough -> y1 partition 96
    W1T[15 + (5 if nkm == 15 else 0) + 2, 96] = 1.0
    W2g = np.zeros((3, 96, 64), np.float32)
    for dh in range(3):
        for g in range(3):
            W2g[dh, g * 32:g * 32 + 32] = w2[dh, g]
    wcat = np.zeros((97, 640), np.float32)
    for s4 in range(4):
        blk = np.zeros((97, 128), np.float32)
        if s4 <= 2:
            blk[0:96, 0:64] = W2g[s4]
        if s4 >= 1:
            blk[0:96, 64:128] = W2g[s4 - 1]
        if s4 == 1:
            blk[96, 0:64] = LARGE
        if s4 == 2:
            blk[96, 64:128] = LARGE
        wcat[:, 128 * s4:128 * s4 + 128] = blk
    wcat[0:15 + nkm, 512:609] = W1T
    biasb = np.zeros((128, 2), np.float32)
    biasb[0:96, 0] = np.tile(np.asarray(b1, np.float32), 3) - LARGE
    biasb[96, 0] = 0.0
    biasb[:, 1] = np.tile(np.asarray(b2, np.float32), 2) - LARGE
    return wcat.ravel().astype(BF16), biasb.ravel().astype(np.float32), nkm


def _prep_xm(x):
    """Per-core padded x+mask planes. x: [B,H,W,1] f32 -> 8 flat bf16 arrays."""
    x = np.asarray(x, np.float32)[..., 0]        # [B,H,W]
    mask = (x != 0.0).astype(np.float32)
    xg = np.zeros((B, H + 4, WP), np.float32)
    mg = np.zeros((B, H + 4, WP), np.float32)
    xg[:, 2:H + 2, 2:W + 2] = x
    mg[:, 2:H + 2, 2:W + 2] = mask
    maps = []
    for c in range(NCORES):
        r0 = c * ROWS
        xm = np.empty((B, 2, SLAB, WP), np.float32)
        xm[:, 0] = xg[:, r0:r0 + SLAB]
        xm[:, 1] = mg[:, r0:r0 + SLAB]
        maps.append(np.concatenate(
            [xm.ravel(), np.zeros(8, np.float32)]).astype(BF16))
    return maps


def kernel(x, bn_gamma, bn_beta, bn_mean, bn_var, w1, b1, w2, b2):
    from concourse.bass_utils import run_bass_kernel_spmd

    wts, biasb, nkm = _prep_consts(bn_gamma, bn_beta, bn_mean, bn_var,
                                   w1, b1, w2, b2)
    if ("nc", nkm) not in _cached:
        _cached[("nc", nkm)] = _build_nc(nkm)
    _cached["nc"] = nc = _cached[("nc", nkm)]
    xms = _prep_xm(x)
    in_maps = [{"xm": xms[c], "wts": wts, "biasd": biasb} for c in range(NCORES)]
    res = run_bass_kernel_spmd(nc, in_maps, list(range(NCORES)))
    full = np.empty((B, 64, H, W), np.float32)
    for c in range(NCORES):
        full[:, :, c * ROWS:(c + 1) * ROWS, :] = (
            np.asarray(res.results[c]["out"], np.float32).reshape(B, 64, ROWS, W))
    return full



# revision 79
# speedup vs baseline: 1.4730x; 1.0005x over previous
"""Trainium2 Bass kernel for submanifold sparse conv net (gnn_message_passing).

Network: mask = (x != 0); y = BN(x) masked; y1 = relu(subm_conv3x3(y, w1) + b1);
y2 = relu(subm_conv3x3(y1, w2) + b2); out = NCHW(y2).  B,H,W = 4,512,512, C: 1->32->64.

Sharding: H split into 8 slabs of 64 rows (one per NeuronCore), 2-row halo.

Per-core design (all per batch; pixel index p = row*516 + col, 2 zero-pad
cols each side; DRAM holds ONE padded x plane + ONE padded mask plane per
batch -- the 15 (dh,dw)-shifted rhs copies are materialized by a single DMA
with a 3-level partition access pattern):

- conv1: one matmul per y1 row: K=30 (2 planes x 3 dh x 5 dw shifted reads),
  M=97 = 3 "dw groups" x 32 channels + 1 mask-passthrough column: group g
  holds y1 evaluated at column+g-1, so conv2 needs only row shifts; column 96
  reproduces the center mask row so conv2's submanifold masking needs no
  separate DMA.
- conv2: TWO output rows per matmul (M=128 = 2 rows x 64 ch): 4 matmuls with
  K=97 (96 y1 partitions + mask row), one per y1 row-shift s in 0..3; lhsT_s
  carries w2[s] in cols 0:64 (row r) and w2[s-1] in cols 64:128 (row r+1).
  This is 2 matmuls/row instead of 3 and yields full-128-partition PSUM
  tiles, halving the relu/copy and output-DMA per-partition traffic.
- Masking (submanifold restrict) folded into matmuls via +LARGE*mask with
  bias -LARGE; BN folded into w1 host-side.
- bf16 matmul operands, fp32 PSUM; relus alternate ACT/DVE (GPSIMD cannot
  read PSUM). Conv1 pairs stream a few rows ahead of conv2 groups across
  chunk boundaries; rhs loads ride the idle Pool/SWDGE queue in 3 pieces
  per plane, out-DMAs are single-row [64,512] on SP/HWDGE (finer grain
  lets input loads slot between them), the final chunk drains odd rows
  via Pool. All knob values (pool bufs, LEAD, split points) were tuned
  against TimelineSim; PE ends ~92% busy at ~183 us vs the ~167 us
  784-matmul floor.
"""

import sys

if "/opt/trn_rl_repo" not in sys.path:
    sys.path.insert(0, "/opt/trn_rl_repo")

import numpy as np
import ml_dtypes

BF16 = ml_dtypes.bfloat16

B, H, W = 4, 512, 512
NCORES = 8
ROWS = H // NCORES          # 64 output rows per core
CHUNK = 32                  # output rows per inner tile
NCHUNK = ROWS // CHUNK
SLAB = ROWS + 4             # 68 input rows incl 2-row halo each side
WP = W + 4                  # 516 padded cols
PLANE = SLAB * WP           # 35088
LROWS = CHUNK + 2           # y1 rows per chunk (1-row halo each side)
LFREE = LROWS * WP          # free elems per rhs1 chunk tile
YF = LROWS * W              # compact y1 free size
LARGE = 256.0
EPS = 1e-5

_cached = {}


def _build_nc(nkm=5):
    # nkm: mask k-rows in conv1's contraction. 5 when the folded BN shift
    # t == 0 (mask taps only carry the LARGE center terms at dh=0), 15 for
    # the general case.
    import concourse.bass as bass
    import concourse.mybir as mybir
    from concourse import bacc, tile

    KC1 = 15 + nkm

    f32 = mybir.dt.float32
    bf16 = mybir.dt.bfloat16
    AP = bass.AP
    Relu = mybir.ActivationFunctionType.Relu
    ADD = mybir.AluOpType.add
    MAX = mybir.AluOpType.max

    nc = bacc.Bacc("TRN2", target_bir_lowering=False, debug=False,
                   num_devices=NCORES)
    # +8 slack: the (plane,dh,dw)-shifted chunk reads run up to 4 elements
    # past the final mask plane (into never-used rhs columns >= 512)
    xm = nc.declare_dram_parameter("xm", [B * 2 * PLANE + 8], bf16, isOutput=False)
    wts = nc.declare_dram_parameter("wts", [97 * 640], bf16, isOutput=False)
    biasd = nc.declare_dram_parameter("biasd", [128 * 2], f32, isOutput=False)
    out = nc.declare_dram_parameter("out", [B * 64 * ROWS * W], f32, isOutput=True)

    with tile.TileContext(nc) as tc:
        with (
            tc.tile_pool(name="const", bufs=1) as cpool,
            tc.tile_pool(name="rhs1", bufs=3) as rpool,
            tc.tile_pool(name="y1", bufs=2) as ypool,
            tc.tile_pool(name="stage", bufs=17) as spool,
            tc.tile_pool(name="ps1", bufs=3, space="PSUM") as p1pool,
            tc.tile_pool(name="ps2", bufs=2, space="PSUM") as p2pool,
        ):
            wcat = cpool.tile([97, 640], bf16, tag="wcat")
            biasb = cpool.tile([128, 2], f32, tag="biasb")
            w1t = wcat[0:KC1, 512:609]
            w2s = [wcat[0:97, 128 * s:128 * s + 128] for s in range(4)]
            bias1 = biasb[0:97, 0:1]
            bias2 = biasb[0:128, 1:2]

            chunks = [(b, k) for b in range(B) for k in range(NCHUNK)]
            NC = len(chunks)
            rtiles = {}
            ytiles = {}
            FA = 18 * WP  # row split so early conv1 pairs start after half a DMA

            def load_rhs(ci):
                # rhs1[(plane,dh,dw), rrl*516+c] = P[plane][32k+rrl+dh, c+dw]
                # (3 partition levels over each padded plane; 2 halves per
                # plane, both planes' first halves first, so the first rows
                # only wait on the leading two transfers)
                bb, kk = chunks[ci]
                rhs1 = rpool.tile([KC1, LFREE], bf16, name=f"rhs_{ci}",
                                  tag="rhs1")
                # both planes' first halves first, so the first pairs only
                # wait on the leading two transfers; issued from the idle
                # Pool engine so they never sit behind out-DMAs on SP
                fend = LFREE if kk == 0 else 32 * WP
                for f0, f1 in ((0, 10 * WP), (10 * WP, FA), (FA, fend)):
                    # k=1 windows shift down 2 rows (tile row 0 = y1 row 33)
                    xbase = bb * 2 * PLANE + (CHUNK + 2) * kk * WP
                    nc.gpsimd.dma_start(
                        out=rhs1[0:15, f0:f1],
                        in_=AP(xm, xbase + f0, [[WP, 3], [1, 5], [1, f1 - f0]]),
                    )
                    if nkm == 15:
                        nc.gpsimd.dma_start(
                            out=rhs1[15:30, f0:f1],
                            in_=AP(xm, xbase + PLANE + f0,
                                   [[WP, 3], [1, 5], [1, f1 - f0]]),
                        )
                    else:
                        nc.gpsimd.dma_start(
                            out=rhs1[15:20, f0:f1],
                            in_=AP(xm, xbase + PLANE + WP + f0,
                                   [[1, 5], [1, f1 - f0]]),
                        )
                rtiles[ci] = rhs1

            def conv1_pair(ci, j):
                # y1 tile rows 2j, 2j+1 of chunk ci
                if j == 0:
                    # y1 compact: rows of 512, partitions 0:96 = 3 col-groups
                    # of 32 ch, partition 96 = center mask row
                    ytiles[ci] = ypool.tile([97, YF], bf16, name=f"y1_{ci}",
                                            tag="y1")
                rhs1 = rtiles[ci]
                y1 = ytiles[ci]
                ps1 = p1pool.tile([97, 1024], f32, tag="ps1")
                for half in range(2):
                    rrl = 2 * j + half
                    nc.tensor.matmul(
                        ps1[:, 512 * half:512 * half + 512], lhsT=w1t,
                        rhs=rhs1[:, rrl * WP:rrl * WP + 512],
                        start=True, stop=True,
                    )
                dst = y1[0:97, 2 * j * W:2 * j * W + 1024]
                if j % 2 == 1:  # GPSIMD can't read PSUM; alternate ACT/DVE
                    nc.scalar.activation(dst, ps1[:, :], Relu, bias=bias1)
                else:
                    nc.vector.tensor_scalar(dst, ps1[:, :], bias1, 0.0,
                                            op0=ADD, op1=MAX)

            # global software pipeline: the conv1 pair stream leads the conv2
            # group stream, flowing across chunk boundaries. k=1 chunks have
            # one pair fewer: their first group reads its two upper y1 rows
            # from the previous chunk's tile (no halo recompute).
            NP = [17 if kk == 0 else 16 for (_, kk) in chunks]
            row_start = [0]
            for n in NP:
                row_start.append(row_start[-1] + 2 * n)
            pair_list = [(ci, j) for ci in range(NC) for j in range(NP[ci])]
            emitted = [0]
            LEAD = 4  # in rows

            def emit_rows_until(target):
                # target is in row units; pairs cover 2 rows each
                while 2 * emitted[0] < min(target, 2 * len(pair_list)):
                    ci, j = pair_list[emitted[0]]
                    conv1_pair(ci, j)
                    emitted[0] += 1

            # chunk 0's leading x-half rides SP so its descriptor generation
            # runs in parallel with Pool's; consts follow on SP
            rhs0 = rpool.tile([KC1, LFREE], bf16, name="rhs_0", tag="rhs1")
            nc.sync.dma_start(
                out=rhs0[0:15, 0:FA],
                in_=AP(xm, 0, [[WP, 3], [1, 5], [1, FA]]),
            )
            if nkm == 15:
                nc.gpsimd.dma_start(
                    out=rhs0[15:30, 0:FA],
                    in_=AP(xm, PLANE, [[WP, 3], [1, 5], [1, FA]]),
                )
            else:
                nc.gpsimd.dma_start(
                    out=rhs0[15:20, 0:FA],
                    in_=AP(xm, PLANE + WP, [[1, 5], [1, FA]]),
                )
            nc.sync.dma_start(out=wcat[:, :], in_=AP(wts, 0, [[640, 97], [1, 640]]))
            nc.sync.dma_start(out=biasb[:, :], in_=AP(biasd, 0, [[2, 128], [1, 2]]))
            for f0, f1 in ((FA, LFREE),):
                nc.gpsimd.dma_start(
                    out=rhs0[0:15, f0:f1],
                    in_=AP(xm, f0, [[WP, 3], [1, 5], [1, f1 - f0]]),
                )
                if nkm == 15:
                    nc.gpsimd.dma_start(
                        out=rhs0[15:30, f0:f1],
                        in_=AP(xm, PLANE + f0,
                               [[WP, 3], [1, 5], [1, f1 - f0]]),
                    )
                else:
                    nc.gpsimd.dma_start(
                        out=rhs0[15:20, f0:f1],
                        in_=AP(xm, PLANE + WP + f0,
                               [[1, 5], [1, f1 - f0]]),
                    )
            rtiles[0] = rhs0
            load_rhs(1)
            for ci, (b, k) in enumerate(chunks):
                if ci + 2 < NC:
                    # prefetch two chunks ahead at chunk top (the row stream
                    # for chunk ci+1 starts now), ahead of this chunk's
                    # out-DMAs in the in-order SP queue
                    load_rhs(ci + 2)
                rtiles.pop(ci - 2, None)
                ytiles.pop(ci - 2, None)
                emit_rows_until(row_start[ci] + 4 + LEAD)
                y1 = ytiles[ci]
                for p in range(CHUNK // 2):
                    ps2 = p2pool.tile([128, 512], f32, tag="ps2")
                    for s in range(4):
                        if k == 0:
                            ysrc, t = y1, 2 * p + s
                        elif p == 0 and s < 2:
                            # top rows live in the previous chunk's tile
                            ysrc, t = ytiles[ci - 1], 32 + s
                        else:
                            ysrc, t = y1, 2 * p + s - 2
                        nc.tensor.matmul(
                            ps2[:, :], lhsT=w2s[s],
                            rhs=ysrc[0:97, t * W:t * W + 512],
                            start=(s == 0), stop=(s == 3),
                        )
                    stage = spool.tile([128, 512], f32, tag="stage")
                    if p % 2 == 0:
                        nc.vector.tensor_scalar(stage[:, :], ps2[:, :], bias2,
                                                0.0, op0=ADD, op1=MAX)
                    else:
                        nc.scalar.activation(stage[:, :], ps2[:, :], Relu,
                                             bias=bias2)
                    # 2 output rows (CHUNK*k + 2p, +1), 64 channels; two
                    # single-row DMAs so input loads can slot in between.
                    # In the final chunk odd rows drain via the idle Pool
                    # SWDGE path to halve the tail flush.
                    for rpar in range(2):
                        eng = (nc.gpsimd if rpar == 1 and ci == NC - 1
                               else nc.sync)
                        eng.dma_start(
                            out=AP(out,
                                   (b * 64 * ROWS + CHUNK * k + 2 * p
                                    + rpar) * W,
                                   [[ROWS * W, 64], [1, W]]),
                            in_=stage[64 * rpar:64 * rpar + 64, :],
                        )
                    emit_rows_until(row_start[ci] + 2 * p + 6 + LEAD)
    nc.finalize()
    return nc


def _prep_consts(bn_gamma, bn_beta, bn_mean, bn_var, w1, b1, w2, b2):
    s = float(bn_gamma[0] / np.sqrt(bn_var[0] + EPS))
    t = float(bn_beta[0] - bn_mean[0] * s)
    w1 = np.asarray(w1, np.float32)  # [3,3,1,32] (kh, kw, ci, co)
    w2 = np.asarray(w2, np.float32)  # [3,3,32,64]
    nkm = 5 if t == 0.0 else 15
    W1T = np.zeros((15 + nkm, 97), np.float32)
    for plane in range(2):
        for dh in (-1, 0, 1):
            if plane == 1 and nkm == 5 and dh != 0:
                continue
            for dw in (-2, -1, 0, 1, 2):
                if plane == 0:
                    kp = (dh + 1) * 5 + (dw + 2)
                else:
                    kp = 15 + ((dh + 1) * 5 if nkm == 15 else 0) + (dw + 2)
                for g in range(3):
                    dwp = dw - (g - 1)
                    col = slice(g * 32, g * 32 + 32)
                    if -1 <= dwp <= 1:
                        coef = s if plane == 0 else t
                        if plane == 0 or nkm == 15:
                            W1T[kp, col] += coef * w1[dh + 1, dwp + 1, 0, :]
                    if plane == 1 and dh == 0 and dw == (g - 1):
                        W1T[kp, col] += LARGE
    # mask center passthrough -> y1 partition 96
    W1T[15 + (5 if nkm == 15 else 0) + 2, 96] = 1.0
    W2g = np.zeros((3, 96, 64), np.float32)
    for dh in range(3):
        for g in range(3):
            W2g[dh, g * 32:g * 32 + 32] = w2[dh, g]
    wcat = np.zeros((97, 640), np.float32)
    for s4 in range(4):
        blk = np.zeros((97, 128), np.float32)
        if s4 <= 2:
            blk[0:96, 0:64] = W2g[s4]
        if s4 >= 1:
            blk[0:96, 64:128] = W2g[s4 - 1]
        if s4 == 1:
            blk[96, 0:64] = LARGE
        if s4 == 2:
            blk[96, 64:128] = LARGE
        wcat[:, 128 * s4:128 * s4 + 128] = blk
    wcat[0:15 + nkm, 512:609] = W1T
    biasb = np.zeros((128, 2), np.float32)
    biasb[0:96, 0] = np.tile(np.asarray(b1, np.float32), 3) - LARGE
    biasb[96, 0] = 0.0
    biasb[:, 1] = np.tile(np.asarray(b2, np.float32), 2) - LARGE
    return wcat.ravel().astype(BF16), biasb.ravel().astype(np.float32), nkm


def _prep_xm(x):
    """Per-core padded x+mask planes. x: [B,H,W,1] f32 -> 8 flat bf16 arrays."""
    x = np.asarray(x, np.float32)[..., 0]        # [B,H,W]
    mask = (x != 0.0).astype(np.float32)
    xg = np.zeros((B, H + 4, WP), np.float32)
    mg = np.zeros((B, H + 4, WP), np.float32)
    xg[:, 2:H + 2, 2:W + 2] = x
    mg[:, 2:H + 2, 2:W + 2] = mask
    maps = []
    for c in range(NCORES):
        r0 = c * ROWS
        xm = np.empty((B, 2, SLAB, WP), np.float32)
        xm[:, 0] = xg[:, r0:r0 + SLAB]
        xm[:, 1] = mg[:, r0:r0 + SLAB]
        maps.append(np.concatenate(
            [xm.ravel(), np.zeros(8, np.float32)]).astype(BF16))
    return maps


def kernel(x, bn_gamma, bn_beta, bn_mean, bn_var, w1, b1, w2, b2):
    from concourse.bass_utils import run_bass_kernel_spmd

    wts, biasb, nkm = _prep_consts(bn_gamma, bn_beta, bn_mean, bn_var,
                                   w1, b1, w2, b2)
    if ("nc", nkm) not in _cached:
        _cached[("nc", nkm)] = _build_nc(nkm)
    _cached["nc"] = nc = _cached[("nc", nkm)]
    xms = _prep_xm(x)
    in_maps = [{"xm": xms[c], "wts": wts, "biasd": biasb} for c in range(NCORES)]
    res = run_bass_kernel_spmd(nc, in_maps, list(range(NCORES)))
    full = np.empty((B, 64, H, W), np.float32)
    for c in range(NCORES):
        full[:, :, c * ROWS:(c + 1) * ROWS, :] = (
            np.asarray(res.results[c]["out"], np.float32).reshape(B, 64, ROWS, W))
    return full



# revision 84
# speedup vs baseline: 1.4798x; 1.0046x over previous
"""Trainium2 Bass kernel for submanifold sparse conv net (gnn_message_passing).

Network: mask = (x != 0); y = BN(x) masked; y1 = relu(subm_conv3x3(y, w1) + b1);
y2 = relu(subm_conv3x3(y1, w2) + b2); out = NCHW(y2).  B,H,W = 4,512,512, C: 1->32->64.

Sharding: H split into 8 slabs of 64 rows (one per NeuronCore), 2-row halo.

Per-core design (all per batch; pixel index p = row*516 + col, 2 zero-pad
cols each side; DRAM holds ONE padded x plane + ONE padded mask plane per
batch -- the 15 (dh,dw)-shifted rhs copies are materialized by a single DMA
with a 3-level partition access pattern):

- conv1: one matmul per y1 row: K=30 (2 planes x 3 dh x 5 dw shifted reads),
  M=97 = 3 "dw groups" x 32 channels + 1 mask-passthrough column: group g
  holds y1 evaluated at column+g-1, so conv2 needs only row shifts; column 96
  reproduces the center mask row so conv2's submanifold masking needs no
  separate DMA.
- conv2: TWO output rows per matmul (M=128 = 2 rows x 64 ch): 4 matmuls with
  K=97 (96 y1 partitions + mask row), one per y1 row-shift s in 0..3; lhsT_s
  carries w2[s] in cols 0:64 (row r) and w2[s-1] in cols 64:128 (row r+1).
  This is 2 matmuls/row instead of 3 and yields full-128-partition PSUM
  tiles, halving the relu/copy and output-DMA per-partition traffic.
- Masking (submanifold restrict) folded into matmuls via +LARGE*mask with
  bias -LARGE; BN folded into w1 host-side.
- bf16 matmul operands, fp32 PSUM; relus alternate ACT/DVE (GPSIMD cannot
  read PSUM). Conv1 pairs stream a few rows ahead of conv2 groups across
  chunk boundaries; rhs loads ride the idle Pool/SWDGE queue in 3 pieces
  per plane, out-DMAs are single-row [64,512] on SP/HWDGE (finer grain
  lets input loads slot between them), the final chunk drains odd rows
  via Pool. All knob values (pool bufs, LEAD, split points) were tuned
  against TimelineSim; PE ends ~92% busy at ~183 us vs the ~167 us
  784-matmul floor.
"""

import sys

if "/opt/trn_rl_repo" not in sys.path:
    sys.path.insert(0, "/opt/trn_rl_repo")

import numpy as np
import ml_dtypes

BF16 = ml_dtypes.bfloat16

B, H, W = 4, 512, 512
NCORES = 8
ROWS = H // NCORES          # 64 output rows per core
CHUNK = 32                  # output rows per inner tile
NCHUNK = ROWS // CHUNK
SLAB = ROWS + 4             # 68 input rows incl 2-row halo each side
WP = W + 4                  # 516 padded cols
PLANE = SLAB * WP           # 35088
LROWS = CHUNK + 2           # y1 rows per chunk (1-row halo each side)
LFREE = LROWS * WP          # free elems per rhs1 chunk tile
YF = LROWS * W              # compact y1 free size
LARGE = 256.0
EPS = 1e-5

_cached = {}


def _build_nc(nkm=5):
    # nkm: mask k-rows in conv1's contraction. 5 when the folded BN shift
    # t == 0 (mask taps only carry the LARGE center terms at dh=0), 15 for
    # the general case.
    import concourse.bass as bass
    import concourse.mybir as mybir
    from concourse import bacc, tile

    KC1 = 15 + nkm

    f32 = mybir.dt.float32
    bf16 = mybir.dt.bfloat16
    AP = bass.AP
    Relu = mybir.ActivationFunctionType.Relu
    ADD = mybir.AluOpType.add
    MAX = mybir.AluOpType.max

    nc = bacc.Bacc("TRN2", target_bir_lowering=False, debug=False,
                   num_devices=NCORES)
    # +8 slack: the (plane,dh,dw)-shifted chunk reads run up to 4 elements
    # past the final mask plane (into never-used rhs columns >= 512)
    xm = nc.declare_dram_parameter("xm", [B * 2 * PLANE + 8], bf16, isOutput=False)
    wts = nc.declare_dram_parameter("wts", [97 * 640], bf16, isOutput=False)
    biasd = nc.declare_dram_parameter("biasd", [128 * 2], f32, isOutput=False)
    out = nc.declare_dram_parameter("out", [B * 64 * ROWS * W], f32, isOutput=True)

    with tile.TileContext(nc) as tc:
        with (
            tc.tile_pool(name="const", bufs=1) as cpool,
            tc.tile_pool(name="rhs1", bufs=3) as rpool,
            tc.tile_pool(name="y1", bufs=2) as ypool,
            tc.tile_pool(name="stage", bufs=17) as spool,
            tc.tile_pool(name="ps1", bufs=3, space="PSUM") as p1pool,
            tc.tile_pool(name="ps2", bufs=2, space="PSUM") as p2pool,
        ):
            wcat = cpool.tile([97, 640], bf16, tag="wcat")
            biasb = cpool.tile([128, 2], f32, tag="biasb")
            w1t = wcat[0:KC1, 512:609]
            w2s = [wcat[0:97, 128 * s:128 * s + 128] for s in range(4)]
            bias1 = biasb[0:97, 0:1]
            bias2 = biasb[0:128, 1:2]

            chunks = [(b, k) for b in range(B) for k in range(NCHUNK)]
            NC = len(chunks)
            rtiles = {}
            ytiles = {}
            FA = 18 * WP  # row split so early conv1 pairs start after half a DMA

            def load_rhs(ci):
                # rhs1[(plane,dh,dw), rrl*516+c] = P[plane][32k+rrl+dh, c+dw]
                # (3 partition levels over each padded plane; 2 halves per
                # plane, both planes' first halves first, so the first rows
                # only wait on the leading two transfers)
                bb, kk = chunks[ci]
                rhs1 = rpool.tile([KC1, LFREE], bf16, name=f"rhs_{ci}",
                                  tag="rhs1")
                # both planes' first halves first, so the first pairs only
                # wait on the leading two transfers; issued from the idle
                # Pool engine so they never sit behind out-DMAs on SP
                fend = LFREE if kk == 0 else 32 * WP
                for f0, f1 in ((0, 10 * WP), (10 * WP, FA), (FA, fend)):
                    # k=1 windows shift down 2 rows (tile row 0 = y1 row 33)
                    xbase = bb * 2 * PLANE + (CHUNK + 2) * kk * WP
                    nc.gpsimd.dma_start(
                        out=rhs1[0:15, f0:f1],
                        in_=AP(xm, xbase + f0, [[WP, 3], [1, 5], [1, f1 - f0]]),
                    )
                    if nkm == 15:
                        nc.gpsimd.dma_start(
                            out=rhs1[15:30, f0:f1],
                            in_=AP(xm, xbase + PLANE + f0,
                                   [[WP, 3], [1, 5], [1, f1 - f0]]),
                        )
                    else:
                        nc.gpsimd.dma_start(
                            out=rhs1[15:20, f0:f1],
                            in_=AP(xm, xbase + PLANE + WP + f0,
                                   [[1, 5], [1, f1 - f0]]),
                        )
                rtiles[ci] = rhs1

            def conv1_pair(ci, j):
                # y1 tile rows 2j, 2j+1 of chunk ci
                if j == 0:
                    # y1 compact: rows of 512, partitions 0:96 = 3 col-groups
                    # of 32 ch, partition 96 = center mask row
                    ytiles[ci] = ypool.tile([97, YF], bf16, name=f"y1_{ci}",
                                            tag="y1")
                rhs1 = rtiles[ci]
                y1 = ytiles[ci]
                ps1 = p1pool.tile([97, 1024], f32, tag="ps1")
                for half in range(2):
                    rrl = 2 * j + half
                    nc.tensor.matmul(
                        ps1[:, 512 * half:512 * half + 512], lhsT=w1t,
                        rhs=rhs1[:, rrl * WP:rrl * WP + 512],
                        start=True, stop=True,
                    )
                dst = y1[0:97, 2 * j * W:2 * j * W + 1024]
                if j < 2:
                    # boundary-critical pairs: relu each row on a different
                    # engine so the chunk's first group waits max() not sum()
                    nc.scalar.activation(y1[0:97, 2 * j * W:2 * j * W + 512],
                                         ps1[:, 0:512], Relu, bias=bias1)
                    nc.vector.tensor_scalar(
                        y1[0:97, (2 * j + 1) * W:(2 * j + 1) * W + 512],
                        ps1[:, 512:1024], bias1, 0.0, op0=ADD, op1=MAX)
                elif j % 2 == 1:  # GPSIMD can't read PSUM; alternate ACT/DVE
                    nc.scalar.activation(dst, ps1[:, :], Relu, bias=bias1)
                else:
                    nc.vector.tensor_scalar(dst, ps1[:, :], bias1, 0.0,
                                            op0=ADD, op1=MAX)

            # global software pipeline: the conv1 pair stream leads the conv2
            # group stream, flowing across chunk boundaries. k=1 chunks have
            # one pair fewer: their first group reads its two upper y1 rows
            # from the previous chunk's tile (no halo recompute).
            NP = [17 if kk == 0 else 16 for (_, kk) in chunks]
            row_start = [0]
            for n in NP:
                row_start.append(row_start[-1] + 2 * n)
            pair_list = [(ci, j) for ci in range(NC) for j in range(NP[ci])]
            emitted = [0]
            LEAD = 4  # in rows

            def emit_rows_until(target):
                # target is in row units; pairs cover 2 rows each
                while 2 * emitted[0] < min(target, 2 * len(pair_list)):
                    ci, j = pair_list[emitted[0]]
                    conv1_pair(ci, j)
                    emitted[0] += 1

            # chunk 0's leading x-half rides SP so its descriptor generation
            # runs in parallel with Pool's; consts follow on SP
            rhs0 = rpool.tile([KC1, LFREE], bf16, name="rhs_0", tag="rhs1")
            nc.sync.dma_start(
                out=rhs0[0:15, 0:FA],
                in_=AP(xm, 0, [[WP, 3], [1, 5], [1, FA]]),
            )
            if nkm == 15:
                nc.gpsimd.dma_start(
                    out=rhs0[15:30, 0:FA],
                    in_=AP(xm, PLANE, [[WP, 3], [1, 5], [1, FA]]),
                )
            else:
                nc.gpsimd.dma_start(
                    out=rhs0[15:20, 0:FA],
                    in_=AP(xm, PLANE + WP, [[1, 5], [1, FA]]),
                )
            nc.sync.dma_start(out=wcat[:, :], in_=AP(wts, 0, [[640, 97], [1, 640]]))
            nc.sync.dma_start(out=biasb[:, :], in_=AP(biasd, 0, [[2, 128], [1, 2]]))
            for f0, f1 in ((FA, LFREE),):
                nc.gpsimd.dma_start(
                    out=rhs0[0:15, f0:f1],
                    in_=AP(xm, f0, [[WP, 3], [1, 5], [1, f1 - f0]]),
                )
                if nkm == 15:
                    nc.gpsimd.dma_start(
                        out=rhs0[15:30, f0:f1],
                        in_=AP(xm, PLANE + f0,
                               [[WP, 3], [1, 5], [1, f1 - f0]]),
                    )
                else:
                    nc.gpsimd.dma_start(
                        out=rhs0[15:20, f0:f1],
                        in_=AP(xm, PLANE + WP + f0,
                               [[1, 5], [1, f1 - f0]]),
                    )
            rtiles[0] = rhs0
            load_rhs(1)
            for ci, (b, k) in enumerate(chunks):
                if ci + 2 < NC:
                    # prefetch two chunks ahead at chunk top (the row stream
                    # for chunk ci+1 starts now), ahead of this chunk's
                    # out-DMAs in the in-order SP queue
                    load_rhs(ci + 2)
                rtiles.pop(ci - 2, None)
                ytiles.pop(ci - 2, None)
                emit_rows_until(row_start[ci] + 4 + LEAD)
                y1 = ytiles[ci]
                for p in range(CHUNK // 2):
                    ps2 = p2pool.tile([128, 512], f32, tag="ps2")
                    for s in range(4):
                        if k == 0:
                            ysrc, t = y1, 2 * p + s
                        elif p == 0 and s < 2:
                            # top rows live in the previous chunk's tile
                            ysrc, t = ytiles[ci - 1], 32 + s
                        else:
                            ysrc, t = y1, 2 * p + s - 2
                        nc.tensor.matmul(
                            ps2[:, :], lhsT=w2s[s],
                            rhs=ysrc[0:97, t * W:t * W + 512],
                            start=(s == 0), stop=(s == 3),
                        )
                    stage = spool.tile([128, 512], f32, tag="stage")
                    if p % 2 == 0:
                        nc.vector.tensor_scalar(stage[:, :], ps2[:, :], bias2,
                                                0.0, op0=ADD, op1=MAX)
                    else:
                        nc.scalar.activation(stage[:, :], ps2[:, :], Relu,
                                             bias=bias2)
                    # 2 output rows (CHUNK*k + 2p, +1), 64 channels; two
                    # single-row DMAs so input loads can slot in between.
                    # In the final chunk odd rows drain via the idle Pool
                    # SWDGE path to halve the tail flush.
                    for rpar in range(2):
                        eng = (nc.gpsimd if rpar == 1 and ci == NC - 1
                               else nc.sync)
                        eng.dma_start(
                            out=AP(out,
                                   (b * 64 * ROWS + CHUNK * k + 2 * p
                                    + rpar) * W,
                                   [[ROWS * W, 64], [1, W]]),
                            in_=stage[64 * rpar:64 * rpar + 64, :],
                        )
                    emit_rows_until(row_start[ci] + 2 * p + 6 + LEAD)
    nc.finalize()
    return nc


def _prep_consts(bn_gamma, bn_beta, bn_mean, bn_var, w1, b1, w2, b2):
    s = float(bn_gamma[0] / np.sqrt(bn_var[0] + EPS))
    t = float(bn_beta[0] - bn_mean[0] * s)
    w1 = np.asarray(w1, np.float32)  # [3,3,1,32] (kh, kw, ci, co)
    w2 = np.asarray(w2, np.float32)  # [3,3,32,64]
    nkm = 5 if t == 0.0 else 15
    W1T = np.zeros((15 + nkm, 97), np.float32)
    for plane in range(2):
        for dh in (-1, 0, 1):
            if plane == 1 and nkm == 5 and dh != 0:
                continue
            for dw in (-2, -1, 0, 1, 2):
                if plane == 0:
                    kp = (dh + 1) * 5 + (dw + 2)
                else:
                    kp = 15 + ((dh + 1) * 5 if nkm == 15 else 0) + (dw + 2)
                for g in range(3):
                    dwp = dw - (g - 1)
                    col = slice(g * 32, g * 32 + 32)
                    if -1 <= dwp <= 1:
                        coef = s if plane == 0 else t
                        if plane == 0 or nkm == 15:
                            W1T[kp, col] += coef * w1[dh + 1, dwp + 1, 0, :]
                    if plane == 1 and dh == 0 and dw == (g - 1):
                        W1T[kp, col] += LARGE
    # mask center passthrough -> y1 partition 96
    W1T[15 + (5 if nkm == 15 else 0) + 2, 96] = 1.0
    W2g = np.zeros((3, 96, 64), np.float32)
    for dh in range(3):
        for g in range(3):
            W2g[dh, g * 32:g * 32 + 32] = w2[dh, g]
    wcat = np.zeros((97, 640), np.float32)
    for s4 in range(4):
        blk = np.zeros((97, 128), np.float32)
        if s4 <= 2:
            blk[0:96, 0:64] = W2g[s4]
        if s4 >= 1:
            blk[0:96, 64:128] = W2g[s4 - 1]
        if s4 == 1:
            blk[96, 0:64] = LARGE
        if s4 == 2:
            blk[96, 64:128] = LARGE
        wcat[:, 128 * s4:128 * s4 + 128] = blk
    wcat[0:15 + nkm, 512:609] = W1T
    biasb = np.zeros((128, 2), np.float32)
    biasb[0:96, 0] = np.tile(np.asarray(b1, np.float32), 3) - LARGE
    biasb[96, 0] = 0.0
    biasb[:, 1] = np.tile(np.asarray(b2, np.float32), 2) - LARGE
    return wcat.ravel().astype(BF16), biasb.ravel().astype(np.float32), nkm


def _prep_xm(x):
    """Per-core padded x+mask planes. x: [B,H,W,1] f32 -> 8 flat bf16 arrays."""
    x = np.asarray(x, np.float32)[..., 0]        # [B,H,W]
    mask = (x != 0.0).astype(np.float32)
    xg = np.zeros((B, H + 4, WP), np.float32)
    mg = np.zeros((B, H + 4, WP), np.float32)
    xg[:, 2:H + 2, 2:W + 2] = x
    mg[:, 2:H + 2, 2:W + 2] = mask
    maps = []
    for c in range(NCORES):
        r0 = c * ROWS
        xm = np.empty((B, 2, SLAB, WP), np.float32)
        xm[:, 0] = xg[:, r0:r0 + SLAB]
        xm[:, 1] = mg[:, r0:r0 + SLAB]
        maps.append(np.concatenate(
            [xm.ravel(), np.zeros(8, np.float32)]).astype(BF16))
    return maps


def kernel(x, bn_gamma, bn_beta, bn_mean, bn_var, w1, b1, w2, b2):
    from concourse.bass_utils import run_bass_kernel_spmd

    wts, biasb, nkm = _prep_consts(bn_gamma, bn_beta, bn_mean, bn_var,
                                   w1, b1, w2, b2)
    if ("nc", nkm) not in _cached:
        _cached[("nc", nkm)] = _build_nc(nkm)
    _cached["nc"] = nc = _cached[("nc", nkm)]
    xms = _prep_xm(x)
    in_maps = [{"xm": xms[c], "wts": wts, "biasd": biasb} for c in range(NCORES)]
    res = run_bass_kernel_spmd(nc, in_maps, list(range(NCORES)))
    full = np.empty((B, 64, H, W), np.float32)
    for c in range(NCORES):
        full[:, :, c * ROWS:(c + 1) * ROWS, :] = (
            np.asarray(res.results[c]["out"], np.float32).reshape(B, 64, ROWS, W))
    return full

